# revision 1
# baseline (speedup 1.0000x reference)
"""APPNP decoder on 8 Trainium2 NeuronCores.

Math (reference):
    src,dst,norm = gcn_norm(edge_index)     # adds self loops, norm = dinv[src]*dinv[dst]
    h1 = x@W3 + b3 ; h1 = appnp(h1) ; h1 = relu(h1)
    h2 = h1@W4 + b4 ; out = appnp(h2)
    appnp: z=h; 10x { z = 0.9*scatter_add(z[src]*norm, dst) + 0.1*h }

Factorized device form (u = dinv * z):
    u_{k+1} = a * (S u_k) + 0.1*u_0        a = 0.9*dinv^2, S = binary adjacency (incl self loops)
    relu commutes with the positive row scale; final out = sqrt(deg) * u.

Distribution: nodes sharded over 8 cores (6250 -> padded 6272 rows/core).
Each hop: AllGather u (bf16) -> every core gathers its in-edge rows with
dma_gather, scatter-adds via one-hot matmuls on the TensorEngine (masks
generated on-the-fly by DVE is_equal), fused DVE epilogue.
"""
import sys
import numpy as np

sys.path.insert(0, '/opt/trn_rl_repo')

N = 50000
NCORES = 8
CIN = 64
C1 = 128
C2 = 64
KHOPS = 10
ALPHA = 0.1
HALF = 32768
TGROUP = 4

_BF16 = None


def _bf16():
    global _BF16
    if _BF16 is None:
        import ml_dtypes
        _BF16 = np.dtype(ml_dtypes.bfloat16)
    return _BF16


# --------------------------------------------------------------------------
# host-side graph preprocessing
# --------------------------------------------------------------------------

class Plan:
    pass


def make_plan(edge_index, n, ncores, half, tgroup, khops, alpha):
    p = Plan()
    rpcr = n // ncores                     # real rows per core
    nt = -(-rpcr // 128)                   # dst tiles per core
    rpc = nt * 128                         # padded rows per core
    npad = rpc * ncores
    assert npad - half < 32768 and half < 32768 + 1, "int16 half split"
    p.n, p.ncores, p.rpcr, p.nt, p.rpc, p.npad = n, ncores, rpcr, nt, rpc, npad
    p.half, p.khops, p.alpha = half, khops, alpha

    src = np.asarray(edge_index[0], dtype=np.int64)
    dst = np.asarray(edge_index[1], dtype=np.int64)
    deg = (np.bincount(dst, minlength=n) + 1).astype(np.float64)
    s, d = src, dst
    dinv = 1.0 / np.sqrt(np.maximum(deg, 1.0))
    p.deg, p.dinv = deg, dinv

    score = s // rpcr
    prow_s = score * rpc + (s - score * rpcr)      # padded global row of src
    dcore = d // rpcr
    ld = d - dcore * rpcr                          # local dst row
    tl = ld >> 7                                   # local tile
    lc = (ld & 127).astype(np.int16)
    h = (prow_s >= half).astype(np.int64)
    idx16 = (prow_s - h * half).astype(np.int16)

    key = (dcore * nt + tl) * 2 + h                # (core, tile, half)
    order = np.argsort(key, kind='stable')
    cnt = np.bincount(key, minlength=ncores * nt * 2).reshape(ncores, nt, 2)
    CC = -(-cnt // 128)
    CC = CC.max(axis=0)                            # [nt, 2] static chunk counts
    p.CC = CC

    # group schedule
    groups = []
    slot = 0
    for g0 in range(0, nt, tgroup):
        tiles = list(range(g0, min(g0 + tgroup, nt)))
        gr = Plan()
        gr.tiles = tiles
        gr.slot_base = slot
        gr.H0 = int(sum(CC[t, 0] for t in tiles))
        gr.H1 = int(sum(CC[t, 1] for t in tiles))
        gr.off0, gr.off1 = {}, {}
        o = 0
        for t in tiles:
            gr.off0[t] = o
            o += int(CC[t, 0])
        o = 0
        for t in tiles:
            gr.off1[t] = o
            o += int(CC[t, 1])
        slot += gr.H0 + gr.H1
        groups.append(gr)
    p.groups = groups
    p.totc = slot
    p.maxH0 = max((g.H0 for g in groups), default=0)
    p.maxH1 = max((g.H1 for g in groups), default=0)
    p.maxH = max((g.H0 + g.H1 for g in groups), default=0)

    # slot base per (tile, half)
    slot_of = np.zeros((nt, 2), np.int64)
    for gr in groups:
        for t in gr.tiles:
            slot_of[t, 0] = gr.slot_base + gr.off0[t]
            slot_of[t, 1] = gr.slot_base + gr.H0 + gr.off1[t]
    p.slot_of = slot_of

    # per-core packed arrays
    p.idx_arrs, p.dstrel_arrs = [], []
    srt_key, srt_idx16, srt_lc = key[order], idx16[order], lc[order]
    bounds = np.searchsorted(srt_key, np.arange(ncores * nt * 2 + 1))
    for m in range(ncores):
        idx_a = np.zeros((16, p.totc * 8), np.int16)
        rel_a = np.full((128, p.totc), 255, np.int16)
        for t in range(nt):
            for hh in range(2):
                k = (m * nt + t) * 2 + hh
                lo, hi = bounds[k], bounds[k + 1]
                if hi == lo:
                    continue
                cnt_e = hi - lo
                base = slot_of[t, hh] * 128
                pos = base + np.arange(cnt_e)
                sl, pp = pos >> 7, pos & 127
                idx_a[pp % 16, sl * 8 + (pp >> 4)] = srt_idx16[lo:hi]
                rel_a[pp, sl] = srt_lc[lo:hi]
        p.idx_arrs.append(np.tile(idx_a, (8, 1)))
        p.dstrel_arrs.append(rel_a)
    return p


def make_inputs(p, x, W3, b3, W4, b4, c1, c2, cin):
    """per-core in_maps (numpy) given plan."""
    bf16 = _bf16()
    dinv32 = p.dinv.astype(np.float64)
    a_full = (1.0 - p.alpha) * dinv32 * dinv32
    sdeg_full = np.sqrt(np.maximum(p.deg, 1.0))
    in_maps = []
    for m in range(p.ncores):
        lo = m * p.rpcr
        rows = np.arange(lo, lo + p.rpcr)

        def padded(vec):
            out = np.zeros(p.rpc, np.float64)
            out[:p.rpcr] = vec[rows]
            return out

        def tiled(vec):      # [rpc] -> [128, nt]
            return vec.reshape(p.nt, 128).T.copy()

        a_t = tiled(padded(a_full)).astype(np.float32)
        dinv_t = tiled(padded(dinv32)).astype(np.float32)
        sdeg_t = tiled(padded(sdeg_full)).astype(np.float32)

        xm = np.zeros((p.rpc, cin), np.float32)
        xm[:p.rpcr] = np.asarray(x[lo:lo + p.rpcr], np.float32)
        xT = np.ascontiguousarray(xm.T).astype(bf16)          # [cin, rpc]

        dv = padded(dinv32)
        db3 = (dv[:, None] * np.asarray(b3, np.float64)[None, :])     # [rpc, c1]
        db4 = (dv[:, None] * np.asarray(b4, np.float64)[None, :])     # [rpc, c2]
        db3_t = db3.reshape(p.nt, 128, c1).transpose(1, 0, 2).reshape(128, p.nt * c1).astype(np.float32)
        db4_t = db4.reshape(p.nt, 128, c2).transpose(1, 0, 2).reshape(128, p.nt * c2).astype(np.float32)

        in_maps.append(dict(
            xT=xT, idx=p.idx_arrs[m], dstrel=p.dstrel_arrs[m],
            a_t=a_t, dinv_t=dinv_t, sdeg_t=sdeg_t, db3=db3_t, db4=db4_t,
        ))
    return in_maps


# --------------------------------------------------------------------------
# bass kernel builder
# --------------------------------------------------------------------------

def build_nc(p, W3, W4, c1, c2, cin, parts=frozenset({'ag', 'gather', 'mm'}),
             gbufs=3, psbufs=6, mask_bf16=False, mkbufs=None):
    import concourse.bass as bass
    import concourse.bacc as bacc
    import concourse.tile as tile
    import concourse.mybir as mybir

    bf16 = _bf16()
    dt = mybir.dt
    AOT = mybir.AluOpType
    nt, rpc, npad, half = p.nt, p.rpc, p.npad, p.half

    nc = bacc.Bacc("TRN2", target_bir_lowering=False, debug=False,
                   num_devices=p.ncores, num_swdge_queues=4)

    # I/O
    xT_d = nc.dram_tensor("xT", [cin, rpc], dt.bfloat16, kind="ExternalInput")
    idx_d = nc.dram_tensor("idx", [128, p.totc * 8], dt.int16, kind="ExternalInput")
    rel_d = nc.dram_tensor("dstrel", [128, p.totc], dt.int16, kind="ExternalInput")
    a_d = nc.dram_tensor("a_t", [128, nt], dt.float32, kind="ExternalInput")
    dinv_d = nc.dram_tensor("dinv_t", [128, nt], dt.float32, kind="ExternalInput")
    sdeg_d = nc.dram_tensor("sdeg_t", [128, nt], dt.float32, kind="ExternalInput")
    db3_d = nc.dram_tensor("db3", [128, nt * c1], dt.float32, kind="ExternalInput")
    db4_d = nc.dram_tensor("db4", [128, nt * c2], dt.float32, kind="ExternalInput")
    out_d = nc.dram_tensor("out", [rpc, c2], dt.float32, kind="ExternalOutput")

    # consts
    W3c = nc.inline_tensor(np.ascontiguousarray(np.asarray(W3, np.float32)).astype(bf16), "W3c")
    W4c = nc.inline_tensor(np.ascontiguousarray(np.asarray(W4, np.float32)).astype(bf16), "W4c")
    iotac = nc.inline_tensor(np.tile(np.arange(128, dtype=np.int16), (128, 1)), "iotac")
    identc = nc.inline_tensor(np.eye(128, dtype=np.float32).astype(bf16), "identc")

    # internal DRAM (collective buffers)
    ccA = nc.dram_tensor("ccA", [rpc, 128], dt.bfloat16)
    ccB = nc.dram_tensor("ccB", [rpc, 128], dt.bfloat16)
    fullA = nc.dram_tensor("fullA", [npad, 128], dt.bfloat16, addr_space="Shared")
    fullB = nc.dram_tensor("fullB", [npad, 128], dt.bfloat16, addr_space="Shared")
    ccS = nc.dram_tensor("ccS", [128, 128], dt.bfloat16)
    fullS = nc.dram_tensor("fullS", [128 * p.ncores, 128], dt.bfloat16, addr_space="Shared")
    RG = [list(range(p.ncores))]

    cc_re = [ccA.ap().rearrange("(t p) c -> p t c", p=128),
             ccB.ap().rearrange("(t p) c -> p t c", p=128)]
    out_re = out_d.ap().rearrange("(t p) c -> p t c", p=128)

    with tile.TileContext(nc) as tc:
        with tc.tile_pool(name="res", bufs=1) as res, \
             tc.tile_pool(name="work", bufs=2) as work, \
             tc.tile_pool(name="gath", bufs=gbufs) as gpool, \
             tc.tile_pool(name="mask", bufs=(mkbufs or gbufs)) as mkpool, \
             tc.tile_pool(name="eptmp", bufs=4) as tpool, \
             tc.tile_pool(name="psum", bufs=psbufs, space="PSUM") as ppool:

            # resident tiles
            idx_s = res.tile([128, p.totc * 8], dt.int16)
            nc.sync.dma_start(idx_s[:, :], idx_d[:, :])
            rel_s = res.tile([128, p.totc], dt.int16)
            nc.sync.dma_start(rel_s[:, :], rel_d[:, :])
            iota_s = res.tile([128, 128], dt.int16)
            nc.sync.dma_start(iota_s[:, :], iotac[:, :])
            a_s = res.tile([128, nt], dt.float32)
            nc.sync.dma_start(a_s[:, :], a_d[:, :])
            dinv_s = res.tile([128, nt], dt.float32)
            nc.sync.dma_start(dinv_s[:, :], dinv_d[:, :])
            sdeg_s = res.tile([128, nt], dt.float32)
            nc.sync.dma_start(sdeg_s[:, :], sdeg_d[:, :])
            stage = res.tile([128, nt, 128], dt.bfloat16)   # u staging (both phases)
            v0 = res.tile([128, nt, c1], dt.bfloat16)       # 0.1*u0 (both phases)
            wbuf = res.tile([128, nt, c1], dt.bfloat16)     # a*u_old + v0, per hop

            def hop_body(ufull, c, mode):
                """one propagation hop reading u from `ufull` into `stage`.
                mode: 'plain' | 'relu' | 'final' (final scales by sdeg into outstage)"""
                qrot = [0]
                # w = a*u_old + v0 off the psum critical path (runs during the AG)
                for t in range(nt):
                    nc.vector.scalar_tensor_tensor(
                        wbuf[:, t, 0:c], stage[:, t, 0:c], a_s[:, t:t + 1],
                        v0[:, t, 0:c], AOT.mult, AOT.add)
                for gr in p.groups:
                    nch = gr.H0 + gr.H1
                    if nch == 0:
                        continue
                    gb0 = gpool.tile([128, p.maxH0, 128], dt.bfloat16, tag="g0")
                    gb1 = gpool.tile([128, p.maxH1, 128], dt.bfloat16, tag="g1")
                    sb = gr.slot_base
                    if 'gather' not in parts:
                        # debug: fake the gathered data with a cheap memset
                        nc.vector.memset(gb0[:, 0:1, :], 0.0)
                        nc.vector.memset(gb1[:, 0:1, :], 0.0)
                    # dma_gather crashes the device above 1024 idxs/call: split
                    # each (group, half) run into <=8-chunk sub-calls.
                    GC = 8
                    if 'gather' in parts:
                        for hh, (Hn, gb, base) in enumerate(
                                [(gr.H0, gb0, 0), (gr.H1, gb1, half)]):
                            soff = sb if hh == 0 else sb + gr.H0
                            for j in range(0, Hn, GC):
                                w = min(GC, Hn - j)
                                nc.gpsimd.dma_gather(
                                    out_ap=gb[:, j:j + w, :],
                                    in_ap=ufull.ap()[base:(half if hh == 0 else npad), :],
                                    idxs_ap=idx_s[:, (soff + j) * 8:(soff + j + w) * 8],
                                    num_idxs=w * 128, num_idxs_reg=w * 128,
                                    elem_size=128,
                                    queue_num=qrot[0] % 4)
                                qrot[0] += 1
                    mk = mkpool.tile([128, p.maxH, 128],
                                     dt.bfloat16 if mask_bf16 else dt.float8e4, tag="mk")
                    if 'mm' in parts and not ('mask1' in parts and gr.slot_base > 0):
                        nc.vector.tensor_tensor(
                            mk[:, 0:nch, :],
                            rel_s[:, sb:sb + nch].unsqueeze(2).broadcast_to([128, nch, 128]),
                            iota_s[:, :].unsqueeze(1).broadcast_to([128, nch, 128]),
                            AOT.is_equal)
                    for t in gr.tiles:
                        cc0, cc1 = int(p.CC[t, 0]), int(p.CC[t, 1])
                        nchunks = cc0 + cc1
                        if nchunks == 0:
                            continue
                        tmp = tpool.tile([128, c1], dt.float32, tag="tmp")
                        if 'mm' not in parts:
                            # debug: consume gathers, fake the aggregation
                            nc.vector.tensor_tensor(tmp[:, 0:c], gb0[:, 0, 0:c],
                                                    gb1[:, 0, 0:c], AOT.add)
                        else:
                            ps = ppool.tile([128, c], dt.float32, tag="ps")
                            step = 4 if 'mmquarter' in parts else 1
                            sched = [(gb0, gr.off0[t] + j, gr.off0[t] + j)
                                     for j in range(cc0)]
                            sched += [(gb1, gr.off1[t] + j, gr.H0 + gr.off1[t] + j)
                                      for j in range(cc1)]
                            sched = sched[::step]
                            for k, (gb_, col, mcol) in enumerate(sched):
                                nc.tensor.matmul(
                                    ps[:, :], lhsT=mk[:, mcol, :],
                                    rhs=gb_[:, col, 0:c],
                                    start=(k == 0), stop=(k == len(sched) - 1))
                        if 'mm' in parts:
                            # u_new = a*psum + w  (one DVE op on the psum path)
                            if mode == 'plain':
                                nc.vector.scalar_tensor_tensor(
                                    stage[:, t, 0:c], ps[:, :], a_s[:, t:t + 1],
                                    wbuf[:, t, 0:c], AOT.mult, AOT.add)
                            elif mode == 'relu':
                                nc.vector.scalar_tensor_tensor(
                                    tmp[:, 0:c], ps[:, :], a_s[:, t:t + 1],
                                    wbuf[:, t, 0:c], AOT.mult, AOT.add)
                                nc.vector.tensor_scalar_max(
                                    stage[:, t, 0:c], tmp[:, 0:c], 0.0)
                            else:   # final
                                nc.vector.scalar_tensor_tensor(
                                    tmp[:, 0:c], ps[:, :], a_s[:, t:t + 1],
                                    wbuf[:, t, 0:c], AOT.mult, AOT.add)
                                nc.vector.tensor_scalar_mul(
                                    outstage[:, t, :], tmp[:, 0:c], sdeg_s[:, t:t + 1])
                        else:
                            nc.vector.tensor_copy(stage[:, t, 0:c], tmp[:, 0:c])

            # ---------------- phase 0: u0 = dinv*(x@W3) + dinv*b3 ----------------
            with tc.tile_pool(name="ph0", bufs=1) as p0:
                xT_s = p0.tile([cin, nt * 128], dt.bfloat16)
                nc.sync.dma_start(xT_s[:, :], xT_d[:, :])
                W3_s = p0.tile([cin, c1], dt.bfloat16)
                nc.sync.dma_start(W3_s[:, :], W3c[:, :])
                db3_s = p0.tile([128, nt * c1], dt.float32)
                nc.sync.dma_start(db3_s[:, :], db3_d[:, :])
                for t in range(nt):
                    ps = ppool.tile([128, c1], dt.float32, tag="ps")
                    nc.tensor.matmul(ps[:, :], lhsT=xT_s[:, t * 128:(t + 1) * 128],
                                     rhs=W3_s[:, :], start=True, stop=True)
                    tmp = tpool.tile([128, c1], dt.float32, tag="tmp")
                    nc.vector.scalar_tensor_tensor(
                        tmp[:, :], ps[:, :], dinv_s[:, t:t + 1],
                        db3_s[:, t * c1:(t + 1) * c1], AOT.mult, AOT.add)
                    nc.vector.tensor_copy(stage[:, t, :], tmp[:, :])
                    nc.vector.tensor_scalar_mul(v0[:, t, :], tmp[:, :], p.alpha)
            nc.sync.dma_start(cc_re[0], stage[:, :, :])

            # ---------------- phase 1 hops ----------------
            for hp in range(p.khops):
                src_cc = [ccA, ccB][hp % 2]
                ufull = [fullA, fullB][hp % 2]
                if 'agsmall' in parts:
                    nc.gpsimd.collective_compute(
                        "AllGather", AOT.bypass, replica_groups=RG,
                        ins=[ccS.ap().opt()], outs=[fullS.ap().opt()])
                elif 'ag' in parts:
                    nc.gpsimd.collective_compute(
                        "AllGather", AOT.bypass, replica_groups=RG,
                        ins=[src_cc.ap().opt()], outs=[ufull.ap().opt()])
                hop_body(ufull, c1, 'relu' if hp == p.khops - 1 else 'plain')
                nc.sync.dma_start(cc_re[(hp + 1) % 2], stage[:, :, :])

            # ---------------- transition: u2_0 = relu_u1 @ W4 + dinv*b4 ----------
            with tc.tile_pool(name="tr", bufs=1) as tr, \
                 tc.tile_pool(name="trw", bufs=2) as trw, \
                 tc.tile_pool(name="pst", bufs=2, space="PSUM") as pst:
                W4_s = tr.tile([c1, c2], dt.bfloat16)
                nc.sync.dma_start(W4_s[:, :], W4c[:, :])
                id_s = tr.tile([128, 128], dt.bfloat16)
                nc.sync.dma_start(id_s[:, :], identc[:, :])
                db4_s = tr.tile([128, nt * c2], dt.float32)
                nc.sync.dma_start(db4_s[:, :], db4_d[:, :])
                for t in range(nt):
                    psT = pst.tile([128, 128], dt.bfloat16, tag="psT")
                    nc.tensor.transpose(psT[:, :], stage[:, t, :], id_s[:, :])
                    uT = trw.tile([128, 128], dt.bfloat16, tag="uT")
                    nc.vector.tensor_copy(uT[:, :], psT[:, :])
                    ps = ppool.tile([128, c1], dt.float32, tag="ps")
                    nc.tensor.matmul(ps[:, 0:c2], lhsT=uT[:, :], rhs=W4_s[:, :],
                                     start=True, stop=True)
                    tmp = tpool.tile([128, c1], dt.float32, tag="tmp")
                    nc.vector.tensor_tensor(tmp[:, 0:c2], ps[:, 0:c2],
                                            db4_s[:, t * c2:(t + 1) * c2], AOT.add)
                    nc.vector.tensor_copy(stage[:, t, 0:c2], tmp[:, 0:c2])
                    nc.vector.tensor_scalar_mul(v0[:, t, 0:c2], tmp[:, 0:c2], p.alpha)
            nc.sync.dma_start(cc_re[0], stage[:, :, :])

            # ---------------- phase 2 hops ----------------
            outstage = res.tile([128, nt, c2], dt.float32)
            for hp in range(p.khops):
                src_cc = [ccA, ccB][hp % 2]
                ufull = [fullA, fullB][hp % 2]
                if 'agsmall' in parts:
                    nc.gpsimd.collective_compute(
                        "AllGather", AOT.bypass, replica_groups=RG,
                        ins=[ccS.ap().opt()], outs=[fullS.ap().opt()])
                elif 'ag' in parts:
                    nc.gpsimd.collective_compute(
                        "AllGather", AOT.bypass, replica_groups=RG,
                        ins=[src_cc.ap().opt()], outs=[ufull.ap().opt()])
                hop_body(ufull, c2, 'final' if hp == p.khops - 1 else 'plain')
                if hp != p.khops - 1:
                    nc.sync.dma_start(cc_re[(hp + 1) % 2], stage[:, :, :])
            nc.sync.dma_start(out_re, outstage[:, :, :])

    nc.compile()
    return nc


# --------------------------------------------------------------------------
# entry point
# --------------------------------------------------------------------------

_CACHE = {}


def _build_and_run(x, edge_index, W3, b3, W4, b4, n, ncores, cin, c1, c2,
                   khops, alpha, half, tgroup, trace=False,
                   parts=frozenset({'ag', 'gather', 'mm'}), gbufs=3):
    from concourse.bass_utils import run_bass_kernel_spmd
    p = make_plan(edge_index, n, ncores, half, tgroup, khops, alpha)
    in_maps = make_inputs(p, x, W3, b3, W4, b4, c1, c2, cin)
    nc = build_nc(p, W3, W4, c1, c2, cin, parts=parts, gbufs=gbufs)
    res = run_bass_kernel_spmd(nc, in_maps, core_ids=list(range(ncores)),
                               trace=trace)
    outs = [res.results[m]["out"][:p.rpcr] for m in range(ncores)]
    full = np.concatenate(outs, axis=0).astype(np.float32)
    return full, res


def kernel(x, edge_index, W3, b3, W4, b4):
    out, _ = _build_and_run(
        np.asarray(x), np.asarray(edge_index), np.asarray(W3), np.asarray(b3),
        np.asarray(W4), np.asarray(b4),
        n=N, ncores=NCORES, cin=CIN, c1=C1, c2=C2, khops=KHOPS, alpha=ALPHA,
        half=HALF, tgroup=TGROUP)
    return out



# revision 8
# speedup vs baseline: 1.0195x; 1.0195x over previous
"""APPNP decoder on 8 Trainium2 NeuronCores.

Math (reference):
    src,dst,norm = gcn_norm(edge_index)     # adds self loops, norm = dinv[src]*dinv[dst]
    h1 = x@W3 + b3 ; h1 = appnp(h1) ; h1 = relu(h1)
    h2 = h1@W4 + b4 ; out = appnp(h2)
    appnp: z=h; 10x { z = 0.9*scatter_add(z[src]*norm, dst) + 0.1*h }

Factorized device form (u = dinv * z):
    u_{k+1} = a * (S u_k) + 0.1*u_0        a = 0.9*dinv^2, S = binary adjacency (incl self loops)
    relu commutes with the positive row scale; final out = sqrt(deg) * u.

Distribution: nodes sharded over 8 cores (6250 -> padded 6272 rows/core).
Each hop: AllGather u (bf16) -> every core gathers its in-edge rows with
dma_gather, scatter-adds via one-hot matmuls on the TensorEngine (masks
generated on-the-fly by DVE is_equal), fused DVE epilogue.
"""
import sys
import numpy as np

sys.path.insert(0, '/opt/trn_rl_repo')

N = 50000
NCORES = 8
CIN = 64
C1 = 128
C2 = 64
KHOPS = 10
ALPHA = 0.1
HALF = 32768
TGROUP = 4

_BF16 = None


def _bf16():
    global _BF16
    if _BF16 is None:
        import ml_dtypes
        _BF16 = np.dtype(ml_dtypes.bfloat16)
    return _BF16


# --------------------------------------------------------------------------
# host-side graph preprocessing
# --------------------------------------------------------------------------

class Plan:
    pass


def make_plan(edge_index, n, ncores, half, tgroup, khops, alpha):
    p = Plan()
    rpcr = n // ncores                     # real rows per core
    nt = -(-rpcr // 128)                   # dst tiles per core
    rpc = nt * 128                         # padded rows per core
    npad = rpc * ncores
    assert npad - half < 32768 and half < 32768 + 1, "int16 half split"
    p.n, p.ncores, p.rpcr, p.nt, p.rpc, p.npad = n, ncores, rpcr, nt, rpc, npad
    p.half, p.khops, p.alpha = half, khops, alpha

    src = np.asarray(edge_index[0], dtype=np.int64)
    dst = np.asarray(edge_index[1], dtype=np.int64)
    deg = (np.bincount(dst, minlength=n) + 1).astype(np.float64)
    s, d = src, dst
    dinv = 1.0 / np.sqrt(np.maximum(deg, 1.0))
    p.deg, p.dinv = deg, dinv

    score = s // rpcr
    prow_s = score * rpc + (s - score * rpcr)      # padded global row of src
    dcore = d // rpcr
    ld = d - dcore * rpcr                          # local dst row
    tl = ld >> 7                                   # local tile
    lc = (ld & 127).astype(np.int16)
    h = (prow_s >= half).astype(np.int64)
    idx16 = (prow_s - h * half).astype(np.int16)

    key = (dcore * nt + tl) * 2 + h                # (core, tile, half)
    order = np.argsort(key, kind='stable')
    cnt = np.bincount(key, minlength=ncores * nt * 2).reshape(ncores, nt, 2)
    CC = -(-cnt // 128)
    CC = CC.max(axis=0)                            # [nt, 2] static chunk counts
    p.CC = CC

    # group schedule
    groups = []
    slot = 0
    for g0 in range(0, nt, tgroup):
        tiles = list(range(g0, min(g0 + tgroup, nt)))
        gr = Plan()
        gr.tiles = tiles
        gr.slot_base = slot
        gr.H0 = int(sum(CC[t, 0] for t in tiles))
        gr.H1 = int(sum(CC[t, 1] for t in tiles))
        gr.off0, gr.off1 = {}, {}
        o = 0
        for t in tiles:
            gr.off0[t] = o
            o += int(CC[t, 0])
        o = 0
        for t in tiles:
            gr.off1[t] = o
            o += int(CC[t, 1])
        slot += gr.H0 + gr.H1
        groups.append(gr)
    p.groups = groups
    p.totc = slot
    p.maxH0 = max((g.H0 for g in groups), default=0)
    p.maxH1 = max((g.H1 for g in groups), default=0)
    p.maxH = max((g.H0 + g.H1 for g in groups), default=0)

    # slot base per (tile, half)
    slot_of = np.zeros((nt, 2), np.int64)
    for gr in groups:
        for t in gr.tiles:
            slot_of[t, 0] = gr.slot_base + gr.off0[t]
            slot_of[t, 1] = gr.slot_base + gr.H0 + gr.off1[t]
    p.slot_of = slot_of

    # per-core packed arrays
    p.idx_arrs, p.dstrel_arrs = [], []
    srt_key, srt_idx16, srt_lc = key[order], idx16[order], lc[order]
    bounds = np.searchsorted(srt_key, np.arange(ncores * nt * 2 + 1))
    for m in range(ncores):
        idx_a = np.zeros((16, p.totc * 8), np.int16)
        rel_a = np.full((128, p.totc), 255, np.int16)
        for t in range(nt):
            for hh in range(2):
                k = (m * nt + t) * 2 + hh
                lo, hi = bounds[k], bounds[k + 1]
                if hi == lo:
                    continue
                cnt_e = hi - lo
                base = slot_of[t, hh] * 128
                pos = base + np.arange(cnt_e)
                sl, pp = pos >> 7, pos & 127
                idx_a[pp % 16, sl * 8 + (pp >> 4)] = srt_idx16[lo:hi]
                rel_a[pp, sl] = srt_lc[lo:hi]
        p.idx_arrs.append(np.tile(idx_a, (8, 1)))
        p.dstrel_arrs.append(rel_a)
    return p


def make_inputs(p, x, W3, b3, W4, b4, c1, c2, cin):
    """per-core in_maps (numpy) given plan."""
    bf16 = _bf16()
    dinv32 = p.dinv.astype(np.float64)
    a_full = (1.0 - p.alpha) * dinv32 * dinv32
    sdeg_full = np.sqrt(np.maximum(p.deg, 1.0))
    in_maps = []
    for m in range(p.ncores):
        lo = m * p.rpcr
        rows = np.arange(lo, lo + p.rpcr)

        def padded(vec):
            out = np.zeros(p.rpc, np.float64)
            out[:p.rpcr] = vec[rows]
            return out

        def tiled(vec):      # [rpc] -> [128, nt]
            return vec.reshape(p.nt, 128).T.copy()

        a_t = tiled(padded(a_full)).astype(np.float32)
        dinv_t = tiled(padded(dinv32)).astype(np.float32)
        sdeg_t = tiled(padded(sdeg_full)).astype(np.float32)

        xm = np.zeros((p.rpc, cin), np.float32)
        xm[:p.rpcr] = np.asarray(x[lo:lo + p.rpcr], np.float32)
        xT = np.ascontiguousarray(xm.T).astype(bf16)          # [cin, rpc]

        dv = padded(dinv32)
        db3 = (dv[:, None] * np.asarray(b3, np.float64)[None, :])     # [rpc, c1]
        db4 = (dv[:, None] * np.asarray(b4, np.float64)[None, :])     # [rpc, c2]
        db3_t = db3.reshape(p.nt, 128, c1).transpose(1, 0, 2).reshape(128, p.nt * c1).astype(np.float32)
        db4_t = db4.reshape(p.nt, 128, c2).transpose(1, 0, 2).reshape(128, p.nt * c2).astype(np.float32)

        in_maps.append(dict(
            xT=xT, idx=p.idx_arrs[m], dstrel=p.dstrel_arrs[m],
            a_t=a_t, dinv_t=dinv_t, sdeg_t=sdeg_t, db3=db3_t, db4=db4_t,
        ))
    return in_maps


# --------------------------------------------------------------------------
# bass kernel builder
# --------------------------------------------------------------------------

def build_nc(p, W3, W4, c1, c2, cin, parts=frozenset({'ag', 'gather', 'mm'}),
             gbufs=3, psbufs=6, mask_bf16=False, mkbufs=None, nqueues=4):
    import concourse.bass as bass
    import concourse.bacc as bacc
    import concourse.tile as tile
    import concourse.mybir as mybir

    bf16 = _bf16()
    dt = mybir.dt
    AOT = mybir.AluOpType
    nt, rpc, npad, half = p.nt, p.rpc, p.npad, p.half

    nc = bacc.Bacc("TRN2", target_bir_lowering=False, debug=False,
                   num_devices=p.ncores, num_swdge_queues=nqueues)

    # I/O
    xT_d = nc.dram_tensor("xT", [cin, rpc], dt.bfloat16, kind="ExternalInput")
    idx_d = nc.dram_tensor("idx", [128, p.totc * 8], dt.int16, kind="ExternalInput")
    rel_d = nc.dram_tensor("dstrel", [128, p.totc], dt.int16, kind="ExternalInput")
    a_d = nc.dram_tensor("a_t", [128, nt], dt.float32, kind="ExternalInput")
    dinv_d = nc.dram_tensor("dinv_t", [128, nt], dt.float32, kind="ExternalInput")
    sdeg_d = nc.dram_tensor("sdeg_t", [128, nt], dt.float32, kind="ExternalInput")
    db3_d = nc.dram_tensor("db3", [128, nt * c1], dt.float32, kind="ExternalInput")
    db4_d = nc.dram_tensor("db4", [128, nt * c2], dt.float32, kind="ExternalInput")
    out_d = nc.dram_tensor("out", [rpc, c2], dt.float32, kind="ExternalOutput")

    # consts
    W3c = nc.inline_tensor(np.ascontiguousarray(np.asarray(W3, np.float32)).astype(bf16), "W3c")
    W4c = nc.inline_tensor(np.ascontiguousarray(np.asarray(W4, np.float32)).astype(bf16), "W4c")
    iotac = nc.inline_tensor(np.tile(np.arange(128, dtype=np.int16), (128, 1)), "iotac")
    identc = nc.inline_tensor(np.eye(128, dtype=np.float32).astype(bf16), "identc")

    # internal DRAM (collective buffers)
    ccA = nc.dram_tensor("ccA", [rpc, 128], dt.bfloat16)
    ccB = nc.dram_tensor("ccB", [rpc, 128], dt.bfloat16)
    fullA = nc.dram_tensor("fullA", [npad, 128], dt.bfloat16, addr_space="Shared")
    fullB = nc.dram_tensor("fullB", [npad, 128], dt.bfloat16, addr_space="Shared")
    ccS = nc.dram_tensor("ccS", [128, 128], dt.bfloat16)
    fullS = nc.dram_tensor("fullS", [128 * p.ncores, 128], dt.bfloat16, addr_space="Shared")
    RG = [list(range(p.ncores))]

    cc_re = [ccA.ap().rearrange("(t p) c -> p t c", p=128),
             ccB.ap().rearrange("(t p) c -> p t c", p=128)]
    out_re = out_d.ap().rearrange("(t p) c -> p t c", p=128)

    with tile.TileContext(nc) as tc:
        with tc.tile_pool(name="res", bufs=1) as res, \
             tc.tile_pool(name="work", bufs=2) as work, \
             tc.tile_pool(name="gath", bufs=gbufs) as gpool, \
             tc.tile_pool(name="mask", bufs=(mkbufs or gbufs)) as mkpool, \
             tc.tile_pool(name="eptmp", bufs=4) as tpool, \
             tc.tile_pool(name="psum", bufs=psbufs, space="PSUM") as ppool:

            # resident tiles
            idx_s = res.tile([128, p.totc * 8], dt.int16)
            nc.sync.dma_start(idx_s[:, :], idx_d[:, :])
            rel_s = res.tile([128, p.totc], dt.int16)
            nc.sync.dma_start(rel_s[:, :], rel_d[:, :])
            iota_s = res.tile([128, 128], dt.int16)
            nc.sync.dma_start(iota_s[:, :], iotac[:, :])
            a_s = res.tile([128, nt], dt.float32)
            nc.sync.dma_start(a_s[:, :], a_d[:, :])
            dinv_s = res.tile([128, nt], dt.float32)
            nc.sync.dma_start(dinv_s[:, :], dinv_d[:, :])
            sdeg_s = res.tile([128, nt], dt.float32)
            nc.sync.dma_start(sdeg_s[:, :], sdeg_d[:, :])
            stage = res.tile([128, nt, 128], dt.bfloat16)   # u staging (both phases)
            v0 = res.tile([128, nt, c1], dt.bfloat16)       # 0.1*u0 (both phases)
            wbuf = res.tile([128, nt, c1], dt.bfloat16)     # a*u_old + v0, per hop

            def hop_body(ufull, c, mode):
                """one propagation hop reading u from `ufull` into `stage`.
                mode: 'plain' | 'relu' | 'final' (final scales by sdeg into outstage)"""
                qrot = [0]
                # w = a*u_old + v0 off the psum critical path (runs during the AG)
                for t in range(nt if 'nowbuf' not in parts else 1):
                    nc.vector.scalar_tensor_tensor(
                        wbuf[:, t, 0:c], stage[:, t, 0:c], a_s[:, t:t + 1],
                        v0[:, t, 0:c], AOT.mult, AOT.add)
                for gr in p.groups:
                    nch = gr.H0 + gr.H1
                    if nch == 0:
                        continue
                    gb0 = gpool.tile([128, p.maxH0, 128], dt.bfloat16, tag="g0")
                    gb1 = gpool.tile([128, p.maxH1, 128], dt.bfloat16, tag="g1")
                    sb = gr.slot_base
                    if 'gather' not in parts:
                        # debug: fake the gathered data with a cheap memset
                        nc.vector.memset(gb0[:, 0:1, :], 0.0)
                        nc.vector.memset(gb1[:, 0:1, :], 0.0)
                    # dma_gather crashes the device above 1024 idxs/call: split
                    # each (group, half) run into <=8-chunk sub-calls.
                    GC = 8
                    if 'gather' in parts:
                        for hh, (Hn, gb, base) in enumerate(
                                [(gr.H0, gb0, 0), (gr.H1, gb1, half)]):
                            soff = sb if hh == 0 else sb + gr.H0
                            for j in range(0, Hn, GC):
                                w = min(GC, Hn - j)
                                nc.gpsimd.dma_gather(
                                    out_ap=gb[:, j:j + w, :],
                                    in_ap=ufull.ap()[base:(half if hh == 0 else npad), :],
                                    idxs_ap=idx_s[:, (soff + j) * 8:(soff + j + w) * 8],
                                    num_idxs=w * 128, num_idxs_reg=w * 128,
                                    elem_size=128,
                                    queue_num=qrot[0] % nqueues)
                                qrot[0] += 1
                    mk = mkpool.tile([128, p.maxH, 128],
                                     dt.bfloat16 if mask_bf16 else dt.float8e4, tag="mk")
                    nch_gen = 1 if 'mkonecol' in parts else nch
                    if 'mm' in parts and not ('mask1' in parts and gr.slot_base > 0):
                        nc.vector.tensor_tensor(
                            mk[:, 0:nch_gen, :],
                            rel_s[:, sb:sb + nch_gen].unsqueeze(2).broadcast_to([128, nch_gen, 128]),
                            iota_s[:, :].unsqueeze(1).broadcast_to([128, nch_gen, 128]),
                            AOT.is_equal)
                    for t in gr.tiles:
                        cc0, cc1 = int(p.CC[t, 0]), int(p.CC[t, 1])
                        nchunks = cc0 + cc1
                        if nchunks == 0:
                            continue
                        tmp = tpool.tile([128, c1], dt.float32, tag="tmp")
                        if 'mm' not in parts:
                            # debug: consume gathers, fake the aggregation
                            nc.vector.tensor_tensor(tmp[:, 0:c], gb0[:, 0, 0:c],
                                                    gb1[:, 0, 0:c], AOT.add)
                        else:
                            ps = ppool.tile([128, c], dt.float32, tag="ps")
                            step = 4 if 'mmquarter' in parts else 1
                            sched = [(gb0, gr.off0[t] + j, gr.off0[t] + j)
                                     for j in range(cc0)]
                            sched += [(gb1, gr.off1[t] + j, gr.H0 + gr.off1[t] + j)
                                      for j in range(cc1)]
                            sched = sched[::step]
                            for k, (gb_, col, mcol) in enumerate(sched):
                                nc.tensor.matmul(
                                    ps[:, :],
                                    lhsT=mk[:, 0 if 'mkonecol' in parts else mcol, :],
                                    rhs=gb_[:, col, 0:c],
                                    start=(k == 0), stop=(k == len(sched) - 1))
                        tw = 0 if 'nowbuf' in parts else t
                        if 'mm' in parts:
                            # u_new = a*psum + w  (one DVE op on the psum path)
                            if mode == 'plain':
                                nc.vector.scalar_tensor_tensor(
                                    stage[:, t, 0:c], ps[:, :], a_s[:, t:t + 1],
                                    wbuf[:, tw, 0:c], AOT.mult, AOT.add)
                            elif mode == 'relu':
                                nc.vector.scalar_tensor_tensor(
                                    tmp[:, 0:c], ps[:, :], a_s[:, t:t + 1],
                                    wbuf[:, tw, 0:c], AOT.mult, AOT.add)
                                nc.vector.tensor_scalar_max(
                                    stage[:, t, 0:c], tmp[:, 0:c], 0.0)
                            else:   # final
                                nc.vector.scalar_tensor_tensor(
                                    tmp[:, 0:c], ps[:, :], a_s[:, t:t + 1],
                                    wbuf[:, tw, 0:c], AOT.mult, AOT.add)
                                nc.vector.tensor_scalar_mul(
                                    outstage[:, t, :], tmp[:, 0:c], sdeg_s[:, t:t + 1])
                        else:
                            nc.vector.tensor_copy(stage[:, t, 0:c], tmp[:, 0:c])

            # ---------------- phase 0: u0 = dinv*(x@W3) + dinv*b3 ----------------
            with tc.tile_pool(name="ph0", bufs=1) as p0:
                xT_s = p0.tile([cin, nt * 128], dt.bfloat16)
                nc.sync.dma_start(xT_s[:, :], xT_d[:, :])
                W3_s = p0.tile([cin, c1], dt.bfloat16)
                nc.sync.dma_start(W3_s[:, :], W3c[:, :])
                db3_s = p0.tile([128, nt * c1], dt.float32)
                nc.sync.dma_start(db3_s[:, :], db3_d[:, :])
                for t in range(nt):
                    ps = ppool.tile([128, c1], dt.float32, tag="ps")
                    nc.tensor.matmul(ps[:, :], lhsT=xT_s[:, t * 128:(t + 1) * 128],
                                     rhs=W3_s[:, :], start=True, stop=True)
                    tmp = tpool.tile([128, c1], dt.float32, tag="tmp")
                    nc.vector.scalar_tensor_tensor(
                        tmp[:, :], ps[:, :], dinv_s[:, t:t + 1],
                        db3_s[:, t * c1:(t + 1) * c1], AOT.mult, AOT.add)
                    nc.vector.tensor_copy(stage[:, t, :], tmp[:, :])
                    nc.vector.tensor_scalar_mul(v0[:, t, :], tmp[:, :], p.alpha)
            nc.sync.dma_start(cc_re[0], stage[:, :, :])

            # ---------------- phase 1 hops ----------------
            for hp in range(p.khops):
                src_cc = [ccA, ccB][hp % 2]
                ufull = [fullA, fullB][hp % 2]
                if 'agsmall' in parts:
                    nc.gpsimd.collective_compute(
                        "AllGather", AOT.bypass, replica_groups=RG,
                        ins=[ccS.ap().opt()], outs=[fullS.ap().opt()])
                elif 'ag' in parts:
                    nc.gpsimd.collective_compute(
                        "AllGather", AOT.bypass, replica_groups=RG,
                        ins=[src_cc.ap().opt()], outs=[ufull.ap().opt()])
                hop_body(ufull, c1, 'relu' if hp == p.khops - 1 else 'plain')
                nc.sync.dma_start(cc_re[(hp + 1) % 2], stage[:, :, :])

            # ---------------- transition: u2_0 = relu_u1 @ W4 + dinv*b4 ----------
            with tc.tile_pool(name="tr", bufs=1) as tr, \
                 tc.tile_pool(name="trw", bufs=2) as trw, \
                 tc.tile_pool(name="pst", bufs=2, space="PSUM") as pst:
                W4_s = tr.tile([c1, c2], dt.bfloat16)
                nc.sync.dma_start(W4_s[:, :], W4c[:, :])
                id_s = tr.tile([128, 128], dt.bfloat16)
                nc.sync.dma_start(id_s[:, :], identc[:, :])
                db4_s = tr.tile([128, nt * c2], dt.float32)
                nc.sync.dma_start(db4_s[:, :], db4_d[:, :])
                for t in range(nt):
                    psT = pst.tile([128, 128], dt.bfloat16, tag="psT")
                    nc.tensor.transpose(psT[:, :], stage[:, t, :], id_s[:, :])
                    uT = trw.tile([128, 128], dt.bfloat16, tag="uT")
                    nc.vector.tensor_copy(uT[:, :], psT[:, :])
                    ps = ppool.tile([128, c1], dt.float32, tag="ps")
                    nc.tensor.matmul(ps[:, 0:c2], lhsT=uT[:, :], rhs=W4_s[:, :],
                                     start=True, stop=True)
                    tmp = tpool.tile([128, c1], dt.float32, tag="tmp")
                    nc.vector.tensor_tensor(tmp[:, 0:c2], ps[:, 0:c2],
                                            db4_s[:, t * c2:(t + 1) * c2], AOT.add)
                    nc.vector.tensor_copy(stage[:, t, 0:c2], tmp[:, 0:c2])
                    nc.vector.tensor_scalar_mul(v0[:, t, 0:c2], tmp[:, 0:c2], p.alpha)
            nc.sync.dma_start(cc_re[0], stage[:, :, :])

            # ---------------- phase 2 hops ----------------
            outstage = res.tile([128, nt, c2], dt.float32)
            for hp in range(p.khops):
                src_cc = [ccA, ccB][hp % 2]
                ufull = [fullA, fullB][hp % 2]
                if 'agsmall' in parts:
                    nc.gpsimd.collective_compute(
                        "AllGather", AOT.bypass, replica_groups=RG,
                        ins=[ccS.ap().opt()], outs=[fullS.ap().opt()])
                elif 'ag' in parts:
                    nc.gpsimd.collective_compute(
                        "AllGather", AOT.bypass, replica_groups=RG,
                        ins=[src_cc.ap().opt()], outs=[ufull.ap().opt()])
                hop_body(ufull, c2, 'final' if hp == p.khops - 1 else 'plain')
                if hp != p.khops - 1:
                    nc.sync.dma_start(cc_re[(hp + 1) % 2], stage[:, :, :])
            nc.sync.dma_start(out_re, outstage[:, :, :])

    nc.compile()
    return nc


# --------------------------------------------------------------------------
# entry point
# --------------------------------------------------------------------------

_CACHE = {}


def _build_and_run(x, edge_index, W3, b3, W4, b4, n, ncores, cin, c1, c2,
                   khops, alpha, half, tgroup, trace=False,
                   parts=frozenset({'ag', 'gather', 'mm'}), gbufs=3):
    from concourse.bass_utils import run_bass_kernel_spmd
    p = make_plan(edge_index, n, ncores, half, tgroup, khops, alpha)
    in_maps = make_inputs(p, x, W3, b3, W4, b4, c1, c2, cin)
    nc = build_nc(p, W3, W4, c1, c2, cin, parts=parts, gbufs=gbufs)
    res = run_bass_kernel_spmd(nc, in_maps, core_ids=list(range(ncores)),
                               trace=trace)
    outs = [res.results[m]["out"][:p.rpcr] for m in range(ncores)]
    full = np.concatenate(outs, axis=0).astype(np.float32)
    return full, res


def kernel(x, edge_index, W3, b3, W4, b4):
    out, _ = _build_and_run(
        np.asarray(x), np.asarray(edge_index), np.asarray(W3), np.asarray(b3),
        np.asarray(W4), np.asarray(b4),
        n=N, ncores=NCORES, cin=CIN, c1=C1, c2=C2, khops=KHOPS, alpha=ALPHA,
        half=HALF, tgroup=TGROUP)
    return out



# revision 12
# speedup vs baseline: 1.2240x; 1.2005x over previous
"""APPNP decoder on 8 Trainium2 NeuronCores.

Math (reference):
    src,dst,norm = gcn_norm(edge_index)     # adds self loops, norm = dinv[src]*dinv[dst]
    h1 = x@W3 + b3 ; h1 = appnp(h1) ; h1 = relu(h1)
    h2 = h1@W4 + b4 ; out = appnp(h2)
    appnp: z=h; 10x { z = 0.9*scatter_add(z[src]*norm, dst) + 0.1*h }

Factorized device form (u = dinv * z):
    u_{k+1} = a * (S u_k) + 0.1*u_0        a = 0.9*dinv^2, S = binary adjacency (incl self loops)
    relu commutes with the positive row scale; final out = sqrt(deg) * u.

Distribution: nodes sharded over 8 cores (6250 -> padded 6272 rows/core).
Each hop: AllGather u (bf16) -> every core gathers its in-edge rows with
dma_gather, scatter-adds via one-hot matmuls on the TensorEngine (masks
generated on-the-fly by DVE is_equal), fused DVE epilogue.

All per-core inputs are packed into a single int16 "blob" arg (the per-call
PJRT/axon dispatch cost scales with the number of input args, ~1 ms each).
"""
import sys
import numpy as np

sys.path.insert(0, '/opt/trn_rl_repo')

N = 50000
NCORES = 8
CIN = 64
C1 = 128
C2 = 64
KHOPS = 10
ALPHA = 0.1
HALF = 32768
TGROUP = 4

_BF16 = None


def _bf16():
    global _BF16
    if _BF16 is None:
        import ml_dtypes
        _BF16 = np.dtype(ml_dtypes.bfloat16)
    return _BF16


# --------------------------------------------------------------------------
# host-side graph preprocessing
# --------------------------------------------------------------------------

class Plan:
    pass


def make_plan(edge_index, n, ncores, half, tgroup, khops, alpha):
    p = Plan()
    rpcr = n // ncores                     # real rows per core
    nt = -(-rpcr // 128)                   # dst tiles per core
    rpc = nt * 128                         # padded rows per core
    npad = rpc * ncores
    assert npad - half < 32768 and half < 32768 + 1, "int16 half split"
    p.n, p.ncores, p.rpcr, p.nt, p.rpc, p.npad = n, ncores, rpcr, nt, rpc, npad
    p.half, p.khops, p.alpha = half, khops, alpha

    src = np.asarray(edge_index[0], dtype=np.int64)
    dst = np.asarray(edge_index[1], dtype=np.int64)
    deg = (np.bincount(dst, minlength=n) + 1).astype(np.float64)
    s, d = src, dst
    dinv = 1.0 / np.sqrt(np.maximum(deg, 1.0))
    p.deg, p.dinv = deg, dinv

    score = s // rpcr
    prow_s = score * rpc + (s - score * rpcr)      # padded global row of src
    dcore = d // rpcr
    ld = d - dcore * rpcr                          # local dst row
    tl = ld >> 7                                   # local tile
    lc = (ld & 127).astype(np.int16)
    h = (prow_s >= half).astype(np.int64)
    idx16 = (prow_s - h * half).astype(np.int16)

    key = (dcore * nt + tl) * 2 + h                # (core, tile, half)
    order = np.argsort(key, kind='stable')
    cnt = np.bincount(key, minlength=ncores * nt * 2).reshape(ncores, nt, 2)
    CC = -(-cnt // 128)
    CC = CC.max(axis=0)                            # [nt, 2] static chunk counts
    p.CC = CC

    # group schedule
    groups = []
    slot = 0
    for g0 in range(0, nt, tgroup):
        tiles = list(range(g0, min(g0 + tgroup, nt)))
        gr = Plan()
        gr.tiles = tiles
        gr.slot_base = slot
        gr.H0 = int(sum(CC[t, 0] for t in tiles))
        gr.H1 = int(sum(CC[t, 1] for t in tiles))
        gr.off0, gr.off1 = {}, {}
        o = 0
        for t in tiles:
            gr.off0[t] = o
            o += int(CC[t, 0])
        o = 0
        for t in tiles:
            gr.off1[t] = o
            o += int(CC[t, 1])
        slot += gr.H0 + gr.H1
        groups.append(gr)
    p.groups = groups
    p.totc = slot
    p.maxH0 = max((g.H0 for g in groups), default=0)
    p.maxH1 = max((g.H1 for g in groups), default=0)
    p.maxH = max((g.H0 + g.H1 for g in groups), default=0)

    # slot base per (tile, half)
    slot_of = np.zeros((nt, 2), np.int64)
    for gr in groups:
        for t in gr.tiles:
            slot_of[t, 0] = gr.slot_base + gr.off0[t]
            slot_of[t, 1] = gr.slot_base + gr.H0 + gr.off1[t]
    p.slot_of = slot_of

    # per-core packed arrays (idx kept un-replicated: [16, totc*8])
    p.idx_arrs, p.dstrel_arrs = [], []
    srt_key, srt_idx16, srt_lc = key[order], idx16[order], lc[order]
    bounds = np.searchsorted(srt_key, np.arange(ncores * nt * 2 + 1))
    for m in range(ncores):
        idx_a = np.zeros((16, p.totc * 8), np.int16)
        rel_a = np.full((128, p.totc), 255, np.int16)
        for t in range(nt):
            for hh in range(2):
                k = (m * nt + t) * 2 + hh
                lo, hi = bounds[k], bounds[k + 1]
                if hi == lo:
                    continue
                cnt_e = hi - lo
                base = slot_of[t, hh] * 128
                pos = base + np.arange(cnt_e)
                sl, pp = pos >> 7, pos & 127
                idx_a[pp % 16, sl * 8 + (pp >> 4)] = srt_idx16[lo:hi]
                rel_a[pp, sl] = srt_lc[lo:hi]
        p.idx_arrs.append(idx_a)
        p.dstrel_arrs.append(rel_a)
    return p


def blob_layout(p, cin):
    """Column layout of the single int16 input blob [128, CB]."""
    L = {}
    o = 0

    def add(name, rows, cols):
        nonlocal o
        L[name] = (o, rows, cols)
        o += cols

    add('idx', 128, p.totc * 8)
    add('rel', 128, p.totc)
    for nm in ('dinv_hi', 'dinv_lo', 'a_hi', 'a_lo', 'sdeg_hi', 'sdeg_lo'):
        add(nm, 128, p.nt)
    add('xT', cin, p.rpc)
    L['_total'] = -(-o // 16) * 16
    return L


def _hl(x64):
    """fp64 -> (bf16 hi, bf16 lo) int16 bit views."""
    bf16 = _bf16()
    hi = x64.astype(bf16)
    lo = (x64 - hi.astype(np.float64)).astype(bf16)
    return hi.view(np.int16), lo.view(np.int16)


def make_inputs(p, x, W3, b3, W4, b4, c1, c2, cin):
    """per-core single-blob in_maps (numpy) given plan."""
    bf16 = _bf16()
    dinv64 = p.dinv.astype(np.float64)
    a_full = (1.0 - p.alpha) * dinv64 * dinv64
    sdeg_full = np.sqrt(np.maximum(p.deg, 1.0))
    L = blob_layout(p, cin)
    CB = L['_total']
    in_maps = []
    for m in range(p.ncores):
        lo = m * p.rpcr
        rows = np.arange(lo, lo + p.rpcr)

        def padded(vec):
            out = np.zeros(p.rpc, np.float64)
            out[:p.rpcr] = vec[rows]
            return out

        def tiled(vec):      # [rpc] -> [128, nt]
            return vec.reshape(p.nt, 128).T.copy()

        blob = np.zeros((128, CB), np.int16)

        def put(name, arr):
            o, r, c = L[name]
            assert arr.shape == (r, c), (name, arr.shape, (r, c))
            blob[:r, o:o + c] = arr

        put('idx', np.tile(p.idx_arrs[m], (8, 1)))
        put('rel', p.dstrel_arrs[m])
        dh, dl = _hl(tiled(padded(dinv64)))
        put('dinv_hi', dh); put('dinv_lo', dl)
        ah, al = _hl(tiled(padded(a_full)))
        put('a_hi', ah); put('a_lo', al)
        sh, sl_ = _hl(tiled(padded(sdeg_full)))
        put('sdeg_hi', sh); put('sdeg_lo', sl_)

        xm = np.zeros((p.rpc, cin), np.float32)
        xm[:p.rpcr] = np.asarray(x[lo:lo + p.rpcr], np.float32)
        xT = np.ascontiguousarray(xm.T).astype(bf16)          # [cin, rpc]
        put('xT', xT.view(np.int16))

        in_maps.append(dict(blob=blob))
    return in_maps


# --------------------------------------------------------------------------
# bass kernel builder
# --------------------------------------------------------------------------

def build_nc(p, W3, b3, W4, b4, c1, c2, cin,
             parts=frozenset({'ag', 'gather', 'mm'}),
             gbufs=3, psbufs=6, mask_bf16=False, mkbufs=None, nqueues=4):
    import concourse.bass as bass
    import concourse.bacc as bacc
    import concourse.tile as tile
    import concourse.mybir as mybir

    bf16 = _bf16()
    dt = mybir.dt
    AOT = mybir.AluOpType
    nt, rpc, npad, half = p.nt, p.rpc, p.npad, p.half
    L = blob_layout(p, cin)
    CB = L['_total']

    nc = bacc.Bacc("TRN2", target_bir_lowering=False, debug=False,
                   num_devices=p.ncores, num_swdge_queues=nqueues)

    # single packed input + single output
    blob_d = nc.dram_tensor("blob", [128, CB], dt.int16, kind="ExternalInput")
    out_d = nc.dram_tensor("out", [rpc, c2], dt.float32, kind="ExternalOutput")

    def bsec(name, dtype=None):
        o, r, c = L[name]
        ap = blob_d[0:r, o:o + c]
        if dtype is not None:
            ap = ap.bitcast(dtype)
        return ap

    # consts
    W3c = nc.inline_tensor(np.ascontiguousarray(np.asarray(W3, np.float32)).astype(bf16), "W3c")
    W4c = nc.inline_tensor(np.ascontiguousarray(np.asarray(W4, np.float32)).astype(bf16), "W4c")
    b3r = nc.inline_tensor(np.tile(np.asarray(b3, np.float32)[None, :], (128, 1)), "b3r")
    b4r = nc.inline_tensor(np.tile(np.asarray(b4, np.float32)[None, :], (128, 1)), "b4r")
    iotac = nc.inline_tensor(np.tile(np.arange(128, dtype=np.int16), (128, 1)), "iotac")
    identc = nc.inline_tensor(np.eye(128, dtype=np.float32).astype(bf16), "identc")

    # internal DRAM (collective buffers)
    ccA = nc.dram_tensor("ccA", [rpc, 128], dt.bfloat16)
    ccB = nc.dram_tensor("ccB", [rpc, 128], dt.bfloat16)
    fullA = nc.dram_tensor("fullA", [npad, 128], dt.bfloat16, addr_space="Shared")
    fullB = nc.dram_tensor("fullB", [npad, 128], dt.bfloat16, addr_space="Shared")
    ccS = nc.dram_tensor("ccS", [128, 128], dt.bfloat16)
    fullS = nc.dram_tensor("fullS", [128 * p.ncores, 128], dt.bfloat16, addr_space="Shared")
    RG = [list(range(p.ncores))]

    cc_re = [ccA.ap().rearrange("(t p) c -> p t c", p=128),
             ccB.ap().rearrange("(t p) c -> p t c", p=128)]
    out_re = out_d.ap().rearrange("(t p) c -> p t c", p=128)

    with tile.TileContext(nc) as tc:
        with tc.tile_pool(name="res", bufs=1) as res, \
             tc.tile_pool(name="work", bufs=2) as work, \
             tc.tile_pool(name="gath", bufs=gbufs) as gpool, \
             tc.tile_pool(name="mask", bufs=(mkbufs or gbufs)) as mkpool, \
             tc.tile_pool(name="eptmp", bufs=4) as tpool, \
             tc.tile_pool(name="psum", bufs=psbufs, space="PSUM") as ppool:

            # resident tiles, loaded from the blob
            idx_s = res.tile([128, p.totc * 8], dt.int16)
            nc.sync.dma_start(idx_s[:, :], bsec('idx'))
            rel_s = res.tile([128, p.totc], dt.int16)
            nc.sync.dma_start(rel_s[:, :], bsec('rel'))
            iota_s = res.tile([128, 128], dt.int16)
            nc.sync.dma_start(iota_s[:, :], iotac[:, :])

            dinv_s = res.tile([128, nt], dt.float32)
            a_s = res.tile([128, nt], dt.float32)
            sdeg_s = res.tile([128, nt], dt.float32)
            with tc.tile_pool(name="ld", bufs=1) as ld:
                for outt, hi_name, lo_name in (
                        (dinv_s, 'dinv_hi', 'dinv_lo'),
                        (a_s, 'a_hi', 'a_lo'),
                        (sdeg_s, 'sdeg_hi', 'sdeg_lo')):
                    hi = ld.tile([128, nt], dt.bfloat16, tag=hi_name)
                    nc.sync.dma_start(hi[:, :], bsec(hi_name, dt.bfloat16))
                    lo2 = ld.tile([128, nt], dt.bfloat16, tag=lo_name)
                    nc.sync.dma_start(lo2[:, :], bsec(lo_name, dt.bfloat16))
                    nc.vector.tensor_tensor(outt[:, :], hi[:, :], lo2[:, :], AOT.add)

            stage = res.tile([128, nt, 128], dt.bfloat16)   # u staging (both phases)
            v0 = res.tile([128, nt, c1], dt.bfloat16)       # 0.1*u0 (both phases)
            wbuf = res.tile([128, nt, c1], dt.bfloat16)     # a*u_old + v0, per hop

            def hop_body(ufull, c, mode):
                """one propagation hop reading u from `ufull` into `stage`.
                mode: 'plain' | 'relu' | 'final' (final scales by sdeg into outstage)"""
                qrot = [0]
                # w = a*u_old + v0 off the psum critical path (runs during the AG)
                for t in range(nt if 'nowbuf' not in parts else 1):
                    nc.vector.scalar_tensor_tensor(
                        wbuf[:, t, 0:c], stage[:, t, 0:c], a_s[:, t:t + 1],
                        v0[:, t, 0:c], AOT.mult, AOT.add)
                for gr in p.groups:
                    nch = gr.H0 + gr.H1
                    if nch == 0:
                        continue
                    gb0 = gpool.tile([128, p.maxH0, 128], dt.bfloat16, tag="g0")
                    gb1 = gpool.tile([128, p.maxH1, 128], dt.bfloat16, tag="g1")
                    sb = gr.slot_base
                    if 'gather' not in parts:
                        # debug: fake the gathered data with a cheap memset
                        nc.vector.memset(gb0[:, 0:1, :], 0.0)
                        nc.vector.memset(gb1[:, 0:1, :], 0.0)
                    # dma_gather crashes the device above 1024 idxs/call: split
                    # each (group, half) run into <=8-chunk sub-calls.
                    GC = 8
                    if 'gather' in parts:
                        for hh, (Hn, gb, base) in enumerate(
                                [(gr.H0, gb0, 0), (gr.H1, gb1, half)]):
                            soff = sb if hh == 0 else sb + gr.H0
                            for j in range(0, Hn, GC):
                                w = min(GC, Hn - j)
                                nc.gpsimd.dma_gather(
                                    out_ap=gb[:, j:j + w, :],
                                    in_ap=ufull.ap()[base:(half if hh == 0 else npad), :],
                                    idxs_ap=idx_s[:, (soff + j) * 8:(soff + j + w) * 8],
                                    num_idxs=w * 128, num_idxs_reg=w * 128,
                                    elem_size=128,
                                    queue_num=qrot[0] % nqueues)
                                qrot[0] += 1
                    mk = mkpool.tile([128, p.maxH, 128],
                                     dt.bfloat16 if mask_bf16 else dt.float8e4, tag="mk")
                    nch_gen = 1 if 'mkonecol' in parts else nch
                    if 'mm' in parts and not ('mask1' in parts and gr.slot_base > 0):
                        nc.vector.tensor_tensor(
                            mk[:, 0:nch_gen, :],
                            rel_s[:, sb:sb + nch_gen].unsqueeze(2).broadcast_to([128, nch_gen, 128]),
                            iota_s[:, :].unsqueeze(1).broadcast_to([128, nch_gen, 128]),
                            AOT.is_equal)
                    for t in gr.tiles:
                        cc0, cc1 = int(p.CC[t, 0]), int(p.CC[t, 1])
                        nchunks = cc0 + cc1
                        if nchunks == 0:
                            continue
                        tmp = tpool.tile([128, c1], dt.float32, tag="tmp")
                        if 'mm' not in parts:
                            # debug: consume gathers, fake the aggregation
                            nc.vector.tensor_tensor(tmp[:, 0:c], gb0[:, 0, 0:c],
                                                    gb1[:, 0, 0:c], AOT.add)
                        else:
                            ps = ppool.tile([128, c], dt.float32, tag="ps")
                            step = 4 if 'mmquarter' in parts else 1
                            sched = [(gb0, gr.off0[t] + j, gr.off0[t] + j)
                                     for j in range(cc0)]
                            sched += [(gb1, gr.off1[t] + j, gr.H0 + gr.off1[t] + j)
                                      for j in range(cc1)]
                            sched = sched[::step]
                            for k, (gb_, col, mcol) in enumerate(sched):
                                nc.tensor.matmul(
                                    ps[:, :],
                                    lhsT=mk[:, 0 if 'mkonecol' in parts else mcol, :],
                                    rhs=gb_[:, col, 0:c],
                                    start=(k == 0), stop=(k == len(sched) - 1))
                        tw = 0 if 'nowbuf' in parts else t
                        if 'mm' in parts:
                            # u_new = a*psum + w  (one DVE op on the psum path)
                            if mode == 'plain':
                                nc.vector.scalar_tensor_tensor(
                                    stage[:, t, 0:c], ps[:, :], a_s[:, t:t + 1],
                                    wbuf[:, tw, 0:c], AOT.mult, AOT.add)
                            elif mode == 'relu':
                                nc.vector.scalar_tensor_tensor(
                                    tmp[:, 0:c], ps[:, :], a_s[:, t:t + 1],
                                    wbuf[:, tw, 0:c], AOT.mult, AOT.add)
                                nc.vector.tensor_scalar_max(
                                    stage[:, t, 0:c], tmp[:, 0:c], 0.0)
                            else:   # final
                                nc.vector.scalar_tensor_tensor(
                                    tmp[:, 0:c], ps[:, :], a_s[:, t:t + 1],
                                    wbuf[:, tw, 0:c], AOT.mult, AOT.add)
                                nc.vector.tensor_scalar_mul(
                                    outstage[:, t, :], tmp[:, 0:c], sdeg_s[:, t:t + 1])
                        else:
                            nc.vector.tensor_copy(stage[:, t, 0:c], tmp[:, 0:c])

            # ---------------- phase 0: u0 = dinv*(x@W3 + b3) ----------------
            with tc.tile_pool(name="ph0", bufs=1) as p0:
                xT_s = p0.tile([cin, nt * 128], dt.bfloat16)
                nc.sync.dma_start(xT_s[:, :], bsec('xT', dt.bfloat16))
                W3_s = p0.tile([cin, c1], dt.bfloat16)
                nc.sync.dma_start(W3_s[:, :], W3c[:, :])
                b3_s = p0.tile([128, c1], dt.float32)
                nc.sync.dma_start(b3_s[:, :], b3r[:, :])
                for t in range(nt):
                    ps = ppool.tile([128, c1], dt.float32, tag="ps")
                    nc.tensor.matmul(ps[:, :], lhsT=xT_s[:, t * 128:(t + 1) * 128],
                                     rhs=W3_s[:, :], start=True, stop=True)
                    t1 = tpool.tile([128, c1], dt.float32, tag="tmp")
                    nc.vector.tensor_tensor(t1[:, :], ps[:, :], b3_s[:, :], AOT.add)
                    tmp = tpool.tile([128, c1], dt.float32, tag="tmp")
                    nc.vector.tensor_scalar_mul(tmp[:, :], t1[:, :], dinv_s[:, t:t + 1])
                    nc.vector.tensor_copy(stage[:, t, :], tmp[:, :])
                    nc.vector.tensor_scalar_mul(v0[:, t, :], tmp[:, :], p.alpha)
            nc.sync.dma_start(cc_re[0], stage[:, :, :])

            # ---------------- phase 1 hops ----------------
            for hp in range(p.khops):
                src_cc = [ccA, ccB][hp % 2]
                ufull = [fullA, fullB][hp % 2]
                if 'agsmall' in parts:
                    nc.gpsimd.collective_compute(
                        "AllGather", AOT.bypass, replica_groups=RG,
                        ins=[ccS.ap().opt()], outs=[fullS.ap().opt()])
                elif 'ag' in parts:
                    nc.gpsimd.collective_compute(
                        "AllGather", AOT.bypass, replica_groups=RG,
                        ins=[src_cc.ap().opt()], outs=[ufull.ap().opt()])
                hop_body(ufull, c1, 'relu' if hp == p.khops - 1 else 'plain')
                nc.sync.dma_start(cc_re[(hp + 1) % 2], stage[:, :, :])

            # ---------------- transition: u2_0 = relu_u1 @ W4 + dinv*b4 ----------
            with tc.tile_pool(name="tr", bufs=1) as tr, \
                 tc.tile_pool(name="trw", bufs=2) as trw, \
                 tc.tile_pool(name="pst", bufs=2, space="PSUM") as pst:
                W4_s = tr.tile([c1, c2], dt.bfloat16)
                nc.sync.dma_start(W4_s[:, :], W4c[:, :])
                id_s = tr.tile([128, 128], dt.bfloat16)
                nc.sync.dma_start(id_s[:, :], identc[:, :])
                b4_s = tr.tile([128, c2], dt.float32)
                nc.sync.dma_start(b4_s[:, :], b4r[:, :])
                for t in range(nt):
                    psT = pst.tile([128, 128], dt.bfloat16, tag="psT")
                    nc.tensor.transpose(psT[:, :], stage[:, t, :], id_s[:, :])
                    uT = trw.tile([128, 128], dt.bfloat16, tag="uT")
                    nc.vector.tensor_copy(uT[:, :], psT[:, :])
                    ps = ppool.tile([128, c1], dt.float32, tag="ps")
                    nc.tensor.matmul(ps[:, 0:c2], lhsT=uT[:, :], rhs=W4_s[:, :],
                                     start=True, stop=True)
                    tmp = tpool.tile([128, c1], dt.float32, tag="tmp")
                    nc.vector.scalar_tensor_tensor(
                        tmp[:, 0:c2], b4_s[:, :], dinv_s[:, t:t + 1],
                        ps[:, 0:c2], AOT.mult, AOT.add)
                    nc.vector.tensor_copy(stage[:, t, 0:c2], tmp[:, 0:c2])
                    nc.vector.tensor_scalar_mul(v0[:, t, 0:c2], tmp[:, 0:c2], p.alpha)
            nc.sync.dma_start(cc_re[0], stage[:, :, :])

            # ---------------- phase 2 hops ----------------
            outstage = res.tile([128, nt, c2], dt.float32)
            for hp in range(p.khops):
                src_cc = [ccA, ccB][hp % 2]
                ufull = [fullA, fullB][hp % 2]
                if 'agsmall' in parts:
                    nc.gpsimd.collective_compute(
                        "AllGather", AOT.bypass, replica_groups=RG,
                        ins=[ccS.ap().opt()], outs=[fullS.ap().opt()])
                elif 'ag' in parts:
                    nc.gpsimd.collective_compute(
                        "AllGather", AOT.bypass, replica_groups=RG,
                        ins=[src_cc.ap().opt()], outs=[ufull.ap().opt()])
                hop_body(ufull, c2, 'final' if hp == p.khops - 1 else 'plain')
                if hp != p.khops - 1:
                    nc.sync.dma_start(cc_re[(hp + 1) % 2], stage[:, :, :])
            nc.sync.dma_start(out_re, outstage[:, :, :])

    nc.compile()
    return nc


# --------------------------------------------------------------------------
# entry point
# --------------------------------------------------------------------------

_CACHE = {}


def _build_and_run(x, edge_index, W3, b3, W4, b4, n, ncores, cin, c1, c2,
                   khops, alpha, half, tgroup, trace=False,
                   parts=frozenset({'ag', 'gather', 'mm'}), gbufs=3):
    from concourse.bass_utils import run_bass_kernel_spmd
    p = make_plan(edge_index, n, ncores, half, tgroup, khops, alpha)
    in_maps = make_inputs(p, x, W3, b3, W4, b4, c1, c2, cin)
    nc = build_nc(p, W3, b3, W4, b4, c1, c2, cin, parts=parts, gbufs=gbufs)
    res = run_bass_kernel_spmd(nc, in_maps, core_ids=list(range(ncores)),
                               trace=trace)
    outs = [res.results[m]["out"][:p.rpcr] for m in range(ncores)]
    full = np.concatenate(outs, axis=0).astype(np.float32)
    return full, res


def kernel(x, edge_index, W3, b3, W4, b4):
    out, _ = _build_and_run(
        np.asarray(x), np.asarray(edge_index), np.asarray(W3), np.asarray(b3),
        np.asarray(W4), np.asarray(b4),
        n=N, ncores=NCORES, cin=CIN, c1=C1, c2=C2, khops=KHOPS, alpha=ALPHA,
        half=HALF, tgroup=TGROUP)
    return out
